# revision 1
# baseline (speedup 1.0000x reference)
"""CRF integration (nn_CRFIntegrationModule) Trainium2 kernel.

One image per NeuronCore (B=8 -> 8 cores). Each direction's 32-step scan is
computed as a single hardware tensor_tensor_scan (infinite-window linear
recurrence) plus a windowed correction:

    A_inf[n] = (A_inf[n-1] + u[n-1]) * t[n-1]        (one DVE scan op)
    T_32[n]  = prod_{j=1..32} t[n-j]                 (5 doubling multiplies)
    A_32     = A_inf - T_32 * shift(A_inf, 32)       (exact windowed sum)

with u = E*D (E = exp(-min(var,5)), D = depth), t = m*exp(+-plog) (m = mask).
aw uses the same structure with (E, m).  Horizontal directions run on a
row-major layout (rows in partitions); vertical directions run on a
PE-transposed layout (cols in partitions), then results are transposed back.
"""
import os
import sys

for _p in ("/opt/trn_rl_repo", "/root/.axon_site/_ro/trn_rl_repo"):
    if os.path.isdir(_p) and _p not in sys.path:
        sys.path.insert(0, _p)
        break

import numpy as np
import concourse.bacc as bacc
import concourse.mybir as mybir
import concourse.tile as tile
from concourse import masks
from concourse.bass_utils import run_bass_kernel_spmd

Alu = mybir.AluOpType
ActF = mybir.ActivationFunctionType
F32 = mybir.dt.float32
I32 = mybir.dt.int32

B, H, W = 8, 352, 1216
R = 32          # MAXRANGE
CLIP = 5.0      # CLIPVARIANCE
PAD = 32

# H-phase geometry: row segments (partitions = rows)
RSEGS = [(0, 128), (128, 128), (256, 96)]          # (row0, height)
FH = W + 2 * PAD + 8                               # 1288 (+8: M[n+33] slack)

# V-phase geometry: transposed layout, 2 chunks x 5 col-segments of <=128 cols
VSEG = H + PAD                                     # 384 per col-seg span
NCS = 5                                            # col-segs per chunk
FV = PAD + NCS * VSEG + 8                          # 1960
VCHUNKS = [(0, 640), (640, 576)]                   # (col0, width)
SROWS = 640                                        # staging row-major stride


def _win_chain(nc, op, dst, t, s1, s2, F):
    """dst[n] = OP_{j=1..32} t[n-j] (left window) via doubling."""
    tt = nc.vector.tensor_tensor
    tt(s1[:, 2:F], t[:, 1:F - 1], t[:, 0:F - 2], op=op)
    tt(s2[:, 4:F], s1[:, 4:F], s1[:, 2:F - 2], op=op)
    tt(s1[:, 8:F], s2[:, 8:F], s2[:, 4:F - 4], op=op)
    tt(s2[:, 16:F], s1[:, 16:F], s1[:, 8:F - 8], op=op)
    tt(dst[:, 32:F], s2[:, 32:F], s2[:, 16:F - 16], op=op)


def _dir_pair(nc, m, mb, p, E0, E1, t_l, t_r, u0, u1, AL, BL, AR, BR,
              TL, TR, M, s1, s2, b1, b2, lo, hi, F):
    """Both directions of one axis on [lo,hi) real region of width-F planes.
    m/E/t/u f32; mb/M/b1/b2 bf16 (mask windows are exact 0/1); p = plog plane.
    Outputs: AL = awd (both dirs summed), BL = aw."""
    v = nc.vector
    # windowed mask product (bf16, exact) and windowed plog sum W32 -> TR;
    # chains + ACT exps go first so the exp results are ready when the
    # corrections need them (no DVE->ACT->DVE stall after the scans)
    _win_chain(nc, Alu.mult, M, mb, b1, b2, F)
    _win_chain(nc, Alu.add, TR, p, s1, s2, F)
    # T_L[n] = M[n]*exp(W32[n]);  T_R[n] = M[n+33]*exp(-W32[n+32])
    nc.scalar.activation(s1[:, lo:hi], TR[:, lo:hi], ActF.Exp)
    nc.scalar.activation(s2[:, lo:hi], TR[:, lo + R:hi + R], ActF.Exp,
                         scale=-1.0)
    v.tensor_mul(TL[:, lo:hi], M[:, lo:hi], s1[:, lo:hi])
    v.tensor_mul(TR[:, lo:hi], M[:, lo + R + 1:hi + R + 1], s2[:, lo:hi])
    v.tensor_tensor_scan(AL[:, lo:hi], u0[:, lo - 1:hi - 1], t_l[:, lo - 1:hi - 1],
                         0.0, op0=Alu.add, op1=Alu.mult)
    v.tensor_tensor_scan(BL[:, lo:hi], E0[:, lo - 1:hi - 1], m[:, lo - 1:hi - 1],
                         0.0, op0=Alu.add, op1=Alu.mult)
    v.tensor_tensor_scan(AR[:, lo:hi][:, ::-1], u1[:, lo + 1:hi + 1][:, ::-1],
                         t_r[:, lo + 1:hi + 1][:, ::-1], 0.0,
                         op0=Alu.add, op1=Alu.mult)
    v.tensor_tensor_scan(BR[:, lo:hi][:, ::-1], E1[:, lo + 1:hi + 1][:, ::-1],
                         m[:, lo + 1:hi + 1][:, ::-1], 0.0,
                         op0=Alu.add, op1=Alu.mult)
    # corrections: X32 = X - T*shift(X, 32); M_R[n] = M_L[n+33]
    v.tensor_mul(s1[:, lo:hi], TL[:, lo:hi], AL[:, lo - R:hi - R])
    v.tensor_sub(AL[:, lo:hi], AL[:, lo:hi], s1[:, lo:hi])
    v.tensor_mul(s2[:, lo:hi], TR[:, lo:hi], AR[:, lo + R:hi + R])
    v.tensor_sub(AR[:, lo:hi], AR[:, lo:hi], s2[:, lo:hi])
    v.tensor_mul(s1[:, lo:hi], M[:, lo:hi], BL[:, lo - R:hi - R])
    v.tensor_sub(BL[:, lo:hi], BL[:, lo:hi], s1[:, lo:hi])
    v.tensor_mul(s2[:, lo:hi], M[:, lo + R + 1:hi + R + 1], BR[:, lo + R:hi + R])
    v.tensor_sub(BR[:, lo:hi], BR[:, lo:hi], s2[:, lo:hi])
    v.tensor_add(AL[:, lo:hi], AL[:, lo:hi], AR[:, lo:hi])
    v.tensor_add(BL[:, lo:hi], BL[:, lo:hi], BR[:, lo:hi])


def build_program():
    nc = bacc.Bacc("TRN2", target_bir_lowering=False, debug=False)

    pred_log = nc.dram_tensor("pred_log", [2, H, W], F32, kind="ExternalInput").ap()
    mask = nc.dram_tensor("mask", [1, H, W], I32, kind="ExternalInput").ap()
    variance = nc.dram_tensor("variance", [4, H, W], F32, kind="ExternalInput").ap()
    depth_cur = nc.dram_tensor("depth_cur", [1, H, W], F32, kind="ExternalInput").ap()
    depth_orig = nc.dram_tensor("depth_orig", [1, H, W], F32, kind="ExternalInput").ap()
    lam = nc.dram_tensor("lam", [1], F32, kind="ExternalInput").ap()
    depthout = nc.dram_tensor("depthout", [1, H, W], F32, kind="ExternalOutput").ap()

    with tile.TileContext(nc, pool_alloc_mode="queue") as tc:
        with tc.tile_pool(name="const", bufs=1) as cp, \
             tc.tile_pool(name="scratch", bufs=1, space="DRAM") as dp, \
             tc.tile_pool(name="psum", bufs=8, space="PSUM") as pp:
            ident = cp.tile([128, 128], F32, tag="ident")
            masks.make_identity(nc, ident[:])
            lam_t = cp.tile([128, 1], F32, tag="lam")
            nc.sync.dma_start(lam_t[:, 0:1], lam.partition_broadcast(128))

            twH = dp.tile([H, W], F32, tag="twH")
            twdH = dp.tile([H, W], F32, tag="twdH")

            _h_phase(nc, tc, pred_log, mask, variance, depth_cur, twH, twdH)
            _v_phase(nc, tc, pp, ident, pred_log, mask, variance, depth_cur,
                     depth_orig, twH, twdH, lam_t, depthout)
    nc.finalize()
    return nc


def _h_phase(nc, tc, pred_log, mask, variance, depth, twH, twdH):
    v = nc.vector
    lo, hi = PAD, PAD + W
    with tc.tile_pool(name="hp", bufs=1) as hp:
        def t_(tag, dt=F32):
            return hp.tile([128, FH], dt, tag=tag, name=tag)

        m = t_("m")
        E0, E1 = t_("E0"), t_("E1")
        tl, tr = t_("tl"), t_("tr")
        u0, u1 = t_("u0"), t_("u1")
        AL, BL, AR, BR = t_("AL"), t_("BL"), t_("AR"), t_("BR")
        TL, TR = t_("TL"), t_("TR")
        s1, s2 = t_("s1"), t_("s2")
        BF16 = mybir.dt.bfloat16
        mb = hp.tile([128, FH], BF16, tag="mb", name="mb")
        M = hp.tile([128, FH], BF16, tag="M", name="M")
        b1 = hp.tile([128, FH], BF16, tag="b1", name="b1")
        b2 = hp.tile([128, FH], BF16, tag="b2", name="b2")
        # zero pads once; real regions are rewritten per segment and pad
        # strips are never written afterwards (win-prod pad writes are 0)
        for t in (m, E0, E1, tl, tr, u0, u1, AL, BL, AR, BR):
            v.memset(t[:, 0:PAD], 0.0)
            v.memset(t[:, hi:FH], 0.0)

        for r0, hs in RSEGS:
            rs = slice(r0, r0 + hs)
            D = hp.tile([128, FH], F32, tag="D", name="D", bufs=2)
            v0 = hp.tile([128, FH], F32, tag="v0", name="v0", bufs=2)
            v1 = hp.tile([128, FH], F32, tag="v1", name="v1", bufs=2)
            ph = hp.tile([128, FH], F32, tag="ph", name="ph", bufs=2)
            v.memset(ph[:, 0:PAD], 0.0)
            v.memset(ph[:, hi:FH], 0.0)
            nc.gpsimd.dma_start(m[0:hs, lo:hi], mask[0, rs, :])  # cast load
            nc.sync.dma_start(D[0:hs, lo:hi], depth[0, rs, :])
            nc.sync.dma_start(v0[0:hs, lo:hi], variance[0, rs, :])
            nc.sync.dma_start(v1[0:hs, lo:hi], variance[1, rs, :])
            nc.sync.dma_start(ph[0:hs, lo:hi], pred_log[0, rs, :])

            if hs < 128:  # stale rows from the previous segment: zero them
                nc.gpsimd.memset(m[hs:128, lo:hi], 0.0)
                nc.gpsimd.memset(ph[hs:128, lo:hi], 0.0)
            v.tensor_scalar_min(v0[0:hs, lo:hi], v0[0:hs, lo:hi], CLIP)
            v.tensor_scalar_min(v1[0:hs, lo:hi], v1[0:hs, lo:hi], CLIP)
            nc.scalar.activation(E0[0:hs, lo:hi], v0[0:hs, lo:hi], ActF.Exp,
                                 scale=-1.0)
            nc.scalar.activation(E1[0:hs, lo:hi], v1[0:hs, lo:hi], ActF.Exp,
                                 scale=-1.0)
            if hs < 128:
                nc.gpsimd.memset(E0[hs:128, lo:hi], 0.0)
                nc.gpsimd.memset(E1[hs:128, lo:hi], 0.0)
            # g+ -> s1, g- -> s2
            nc.scalar.activation(s1[0:hs, lo:hi], ph[0:hs, lo:hi], ActF.Exp)
            nc.scalar.activation(s2[0:hs, lo - 1:hi], ph[0:hs, lo - 1:hi],
                                 ActF.Exp, scale=-1.0)
            v.tensor_mul(tl[:, lo:hi], m[:, lo:hi], s1[:, lo:hi])
            v.tensor_mul(tr[:, lo:hi], m[:, lo:hi], s2[:, lo - 1:hi - 1])
            v.tensor_mul(u0[0:hs, lo:hi], E0[0:hs, lo:hi], D[0:hs, lo:hi])
            v.tensor_mul(u1[0:hs, lo:hi], E1[0:hs, lo:hi], D[0:hs, lo:hi])
            if hs < 128:
                nc.gpsimd.memset(u0[hs:128, lo:hi], 0.0)
                nc.gpsimd.memset(u1[hs:128, lo:hi], 0.0)
            v.tensor_copy(mb[:], m[:])   # bf16 mask copy (exact)

            _dir_pair(nc, m, mb, ph, E0, E1, tl, tr, u0, u1, AL, BL, AR, BR,
                      TL, TR, M, s1, s2, b1, b2, lo, hi, FH)

            nc.sync.dma_start(twdH[rs, :], AL[0:hs, lo:hi])
            nc.sync.dma_start(twH[rs, :], BL[0:hs, lo:hi])


def _transpose_plane_in(nc, pp, ident, stag, dst, cw):
    """stag [128, (seg,640)] row-major staging -> dst [128, FV] transposed.
    Full-width col-seg pairs share one PSUM tile and one merged ACT copy
    (3D dest AP, seg stride VSEG) to halve the ACT copy burst."""
    ncs = (cw + 127) // 128
    for rp, (r0, hs) in enumerate(RSEGS):
        cs = 0
        while cs < ncs:
            bw = min(128, cw - cs * 128)
            c = rp * SROWS + cs * 128
            fb = PAD + cs * VSEG + rp * 128
            ng = 0
            while (cs + ng < ncs and ng < 4
                   and min(128, cw - (cs + ng) * 128) == 128):
                ng += 1
            if ng >= 2:
                ps = pp.tile([128, 128 * ng], F32, tag="pt2", bufs=5,
                             name="psg")
                for g in range(ng):
                    nc.tensor.transpose(ps[:, 128 * g:128 * g + hs],
                                        stag[0:hs, c + 128 * g:c + 128 * (g + 1)],
                                        ident[0:hs, 0:hs])
                src = ps[:, 0:128 * ng].rearrange(
                    "p (s c) -> p s c", s=ng)[:, :, 0:hs]
                d = dst[:, fb:fb + VSEG * (ng - 1) + 384].rearrange(
                    "p (s c) -> p s c", s=ng)[:, :, 0:hs]
                nc.scalar.copy(d, src)
                cs += ng
            else:
                ps = pp.tile([128, 128], F32, tag="pt", bufs=3)
                nc.tensor.transpose(ps[0:bw, 0:hs], stag[0:hs, c:c + bw],
                                    ident[0:hs, 0:hs])
                nc.scalar.copy(dst[0:bw, fb:fb + hs], ps[0:bw, 0:hs])
                cs += 1


def _transpose_plane_out(nc, pp, ident, src, stag, cw):
    """src [128, FV] transposed layout -> stag [128, (seg,640)] row-major.
    Full-width col-seg pairs merge into one PSUM tile + one contiguous
    256-wide ACT copy."""
    ncs = (cw + 127) // 128
    for rp, (r0, hs) in enumerate(RSEGS):
        cs = 0
        while cs < ncs:
            bw = min(128, cw - cs * 128)
            fb = PAD + cs * VSEG + rp * 128
            c = rp * SROWS + cs * 128
            ng = 0
            while (cs + ng < ncs and ng < 4
                   and min(128, cw - (cs + ng) * 128) == 128):
                ng += 1
            if ng >= 2:
                ps = pp.tile([128, 128 * ng], F32, tag="pt2", bufs=5,
                             name="psg")
                for g in range(ng):
                    nc.tensor.transpose(ps[0:hs, 128 * g:128 * (g + 1)],
                                        src[:, fb + VSEG * g:fb + VSEG * g + hs],
                                        ident[:, :])
                nc.scalar.copy(stag[0:hs, c:c + 128 * ng],
                               ps[0:hs, 0:128 * ng])
                cs += ng
            else:
                ps = pp.tile([128, 128], F32, tag="pt", bufs=3)
                nc.tensor.transpose(ps[0:hs, 0:bw], src[0:bw, fb:fb + hs],
                                    ident[0:bw, 0:bw])
                nc.scalar.copy(stag[0:hs, c:c + bw], ps[0:hs, 0:bw])
                cs += 1


def _stage_load(nc, stag, dram_plane, c0, cw, gp=False):
    """DRAM [H, W] cols [c0,c0+cw) -> staging [128, (seg,640)] row-major.
    gp=True routes through SWDGE (gpsimd), which casts dtypes in-flight."""
    eng = nc.gpsimd if gp else nc.sync
    eng.dma_start(
        stag[:, 0:2 * SROWS].rearrange("p (s c) -> p s c", s=2)[:, :, 0:cw],
        dram_plane[0:256, c0:c0 + cw].rearrange("(s p) c -> p s c", p=128))
    eng.dma_start(stag[0:96, 2 * SROWS:2 * SROWS + cw],
                  dram_plane[256:352, c0:c0 + cw])


def _stage_store(nc, stag, dram_plane, c0, cw):
    nc.sync.dma_start(
        dram_plane[0:256, c0:c0 + cw].rearrange("(s p) c -> p s c", p=128),
        stag[:, 0:2 * SROWS].rearrange("p (s c) -> p s c", s=2)[:, :, 0:cw])
    nc.sync.dma_start(dram_plane[256:352, c0:c0 + cw],
                      stag[0:96, 2 * SROWS:2 * SROWS + cw])


def _v_phase(nc, tc, pp, ident, pred_log, mask, variance, depth,
             depth_orig, twH, twdH, lam_t, depthout):
    v = nc.vector
    with tc.tile_pool(name="vp", bufs=1) as vp:
        def t_(tag, dt=F32):
            return vp.tile([128, FV], dt, tag=tag, name=tag)

        mT, DT = t_("mT"), t_("DT")
        E2, E3, pv = t_("E2"), t_("E3"), t_("pv")
        tu, td = t_("tu"), t_("td")
        uu, ud = t_("uu"), t_("ud")
        AL, BL, AR, BR = t_("vAL"), t_("vBL"), t_("vAR"), t_("vBR")
        TL, TR = t_("vTL"), t_("vTR")
        s1, s2 = t_("vs1"), t_("vs2")
        BF16 = mybir.dt.bfloat16
        mb = vp.tile([128, FV], BF16, tag="vmb", name="vmb")
        M = vp.tile([128, FV], BF16, tag="vM", name="vM")
        b1 = vp.tile([128, FV], BF16, tag="vb1", name="vb1")
        b2 = vp.tile([128, FV], BF16, tag="vb2", name="vb2")
        rmw = vp.tile([128, 3 * SROWS], F32, tag="rmw")
        rmwd = vp.tile([128, 3 * SROWS], F32, tag="rmwd")

        # zero everything once (on GPSIMD: off the critical DVE);
        # pads/dead regions stay zero afterwards
        for t in (mT, DT, E2, E3, pv, tu, td, uu, ud, AL, BL, AR, BR, s1, s2,
                  rmw, rmwd):
            nc.gpsimd.memset(t[:], 0.0)

        for c0, cw in VCHUNKS:
            with tc.tile_pool(name="vstage", bufs=1) as sp:
                smf = sp.tile([128, 3 * SROWS], F32, tag="smf")
                sst = sp.tile([128, 3 * SROWS], F32, tag="sst")
                nc.gpsimd.memset(smf[:], 0)
                _stage_load(nc, smf, mask[0], c0, cw, gp=True)  # cast load
                _transpose_plane_in(nc, pp, ident, smf, mT, cw)
                _stage_load(nc, sst, depth[0], c0, cw)
                _transpose_plane_in(nc, pp, ident, sst, DT, cw)
                _stage_load(nc, smf, variance[2], c0, cw)
                _transpose_plane_in(nc, pp, ident, smf, E2, cw)
                _stage_load(nc, sst, variance[3], c0, cw)
                _transpose_plane_in(nc, pp, ident, sst, E3, cw)
                _stage_load(nc, smf, pred_log[1], c0, cw)
                _transpose_plane_in(nc, pp, ident, smf, pv, cw)

            ncs = (cw + 127) // 128
            vhi = PAD + (ncs - 1) * VSEG + H   # end of last real region
            # stale cols when cw isn't a multiple of 128 (chunk 1: 64-wide
            # last col-seg): zero partitions [bw,128) of that segment span
            lbw = cw - (ncs - 1) * 128
            if lbw < 128:
                fb = PAD + (ncs - 1) * VSEG
                for t in (mT, DT, E2, E3, pv):
                    v.memset(t[lbw:128, fb:fb + H], 0.0)

            v.tensor_scalar_min(E2[:, PAD:vhi], E2[:, PAD:vhi], CLIP)
            v.tensor_scalar_min(E3[:, PAD:vhi], E3[:, PAD:vhi], CLIP)
            nc.scalar.activation(E2[:, PAD:vhi], E2[:, PAD:vhi], ActF.Exp,
                                 scale=-1.0)
            nc.scalar.activation(E3[:, PAD:vhi], E3[:, PAD:vhi], ActF.Exp,
                                 scale=-1.0)
            nc.scalar.activation(s1[:, PAD:vhi], pv[:, PAD:vhi], ActF.Exp)
            nc.scalar.activation(s2[:, PAD - 1:vhi], pv[:, PAD - 1:vhi],
                                 ActF.Exp, scale=-1.0)
            v.tensor_mul(tu[:, PAD:vhi], mT[:, PAD:vhi], s1[:, PAD:vhi])
            v.tensor_mul(td[:, PAD:vhi], mT[:, PAD:vhi], s2[:, PAD - 1:vhi - 1])
            v.tensor_mul(uu[:, PAD:vhi], E2[:, PAD:vhi], DT[:, PAD:vhi])
            v.tensor_mul(ud[:, PAD:vhi], E3[:, PAD:vhi], DT[:, PAD:vhi])
            # E2/E3 pad strips hold exp(0)=1 after the activation; zero them
            # (they are read as B-scan data0 across segment boundaries --
            # harmless since m=0 there -- but also by u=E*D? DT=0 there, so
            # only the A/B scan "first pad column" tail sees them; windowed
            # T/M = 0 kills those. Zero anyway for the u-planes' sake.)
            for s in range(1, ncs):
                g0 = s * VSEG
                for t in (tu, td, uu, ud, E2, E3):
                    v.memset(t[:, g0:g0 + PAD], 0.0)

            v.tensor_copy(mb[:], mT[:])   # bf16 mask copy (exact)
            _dir_pair(nc, mT, mb, pv, E2, E3, tu, td, uu, ud, AL, BL, AR, BR,
                      TL, TR, M, s1, s2, b1, b2, PAD, vhi, FV)

            _transpose_plane_out(nc, pp, ident, AL, rmwd, cw)
            _transpose_plane_out(nc, pp, ident, BL, rmw, cw)

            # ---- fused final blend for this column chunk (row-major
            # staging layout [128,(seg,640)]); reuses dead V tile slots ----
            FS = 3 * SROWS
            fm = vp.tile([128, FS], F32, tag="vTR", name="fm")
            fDo = vp.tile([128, FS], F32, tag="tu", name="fDo")
            ftw = vp.tile([128, FS], F32, tag="td", name="ftw")
            ftwd = vp.tile([128, FS], F32, tag="uu", name="ftwd")
            fsel = vp.tile([128, FS], F32, tag="ud", name="fsel")
            frc = vp.tile([128, FS], F32, tag="vAR", name="frc")
            fwr = vp.tile([128, FS], F32, tag="vBR", name="fwr")
            for t in (fm, fDo, ftw, ftwd):   # loads only cover [0:cw] slots
                nc.gpsimd.memset(t[:], 0)
            _stage_load(nc, fm, mask[0], c0, cw, gp=True)  # cast load
            _stage_load(nc, fDo, depth_orig[0], c0, cw)
            _stage_load(nc, ftw, twH, c0, cw)
            _stage_load(nc, ftwd, twdH, c0, cw)

            v.tensor_add(ftw[:], ftw[:], rmw[:])              # tw = H + V
            v.tensor_add(ftwd[:], ftwd[:], rmwd[:])
            v.tensor_scalar(fsel[:], ftw[:], 0.0, None, op0=Alu.is_gt)
            v.tensor_mul(fsel[:], fsel[:], fm[:])             # sel = (tw>0)*m
            v.tensor_scalar_max(ftw[:], ftw[:], 1e-6)
            nc.scalar.activation(frc[:], ftw[:], ActF.Ln)
            nc.scalar.activation(frc[:], frc[:], ActF.Exp, scale=-1.0)
            v.tensor_mul(fwr[:], ftw[:], frc[:])              # Newton step
            nc.scalar.activation(fwr[:], fwr[:], ActF.Copy, bias=2.0,
                                 scale=-1.0)
            v.tensor_mul(fwr[:], fwr[:], frc[:])
            v.tensor_mul(ftwd[:], ftwd[:], fwr[:])            # lat = twd*r
            v.tensor_sub(ftwd[:], ftwd[:], fDo[:])
            nc.scalar.activation(fsel[:], fsel[:], ActF.Copy,
                                 scale=lam_t[:, 0:1])         # sel *= lam
            v.tensor_mul(ftwd[:], ftwd[:], fsel[:])
            v.tensor_add(ftwd[:], ftwd[:], fDo[:])
            _stage_store(nc, ftwd, depthout[0], c0, cw)


_NC = None


def _get_nc():
    global _NC
    if _NC is None:
        _NC = build_program()
    return _NC


def kernel(pred_log, mask, variance, depthin, lam, times):
    pred_log = np.ascontiguousarray(np.asarray(pred_log, dtype=np.float32))
    mask = np.ascontiguousarray(np.asarray(mask, dtype=np.int32))
    variance = np.ascontiguousarray(np.asarray(variance, dtype=np.float32))
    depthin = np.ascontiguousarray(np.asarray(depthin, dtype=np.float32))
    lam = np.ascontiguousarray(np.asarray(lam, dtype=np.float32)).reshape(1)
    t = int(np.asarray(times))

    if t <= 0:
        return depthin.copy()
    nc = _get_nc()
    depth_cur = depthin
    for _ in range(t):
        in_maps = [{
            "pred_log": pred_log[b],
            "mask": mask[b],
            "variance": variance[b],
            "depth_cur": depth_cur[b],
            "depth_orig": depthin[b],
            "lam": lam,
        } for b in range(B)]
        res = run_bass_kernel_spmd(nc, in_maps, list(range(B)))
        depth_cur = np.stack([res.results[i]["depthout"] for i in range(B)])
    return depth_cur.astype(np.float32)



# revision 5
# speedup vs baseline: 1.8181x; 1.8181x over previous
"""CRF integration (nn_CRFIntegrationModule) Trainium2 kernel — v2.

One image per NeuronCore (B=8 -> 8 cores). Each direction's 32-step scan is
a hardware tensor_tensor_scan (fp32 carry, bf16 i/o) plus a windowed
correction:

    A_inf[n] = (A_inf[n-1] + u[n-1]) * t[n-1]        (one DVE scan op)
    T_32[n]  = prod_{j=1..32} t[n-j]                 (5 doubling multiplies)
    A_32     = A_inf - T_32 * shift(A_inf, 32)       (exact windowed sum)

v2 layout strategy: the host pre-packs BOTH layouts (row-major segments for
the horizontal scans, transposed column-chunks for the vertical scans) as
bf16 DRAM tensors with pads baked in, so the device does no staging
transposes on the input side.  All elementwise work runs in bf16 (2x DVE
tensor_tensor, 4x tensor_scalar); only the V-phase results are transposed
back on-chip (PE + ACT), and the final blend reads the H-phase tiles
directly from SBUF.  tolerance budget 2e-2 >> bf16 rounding.
"""
import os
import sys

for _p in ("/opt/trn_rl_repo", "/root/.axon_site/_ro/trn_rl_repo"):
    if os.path.isdir(_p) and _p not in sys.path:
        sys.path.insert(0, _p)
        break

import numpy as np
import ml_dtypes
import concourse.bacc as bacc
import concourse.mybir as mybir
import concourse.tile as tile
from concourse import masks
from concourse.bass_utils import run_bass_kernel_spmd

Alu = mybir.AluOpType
ActF = mybir.ActivationFunctionType
F32 = mybir.dt.float32
F16 = mybir.dt.float16
NF16 = np.float16

B, H, W = 8, 352, 1216
R = 32          # MAXRANGE
CLIP = 5.0      # CLIPVARIANCE
PAD = 32
EMIN = float(np.exp(-CLIP))

# H-phase geometry: row segments (partitions = rows), row-major free axis
RSEGS = [(0, 128), (128, 128), (256, 96)]
FH = W + 2 * PAD + 8                               # 1288 (+8: M[n+33] slack)

# V-phase geometry: transposed layout, 2 chunks x 5 col-segments of <=128 cols
VSEG = H + PAD                                     # 384 per col-seg span
NCS = 5
FV = PAD + NCS * VSEG + 8                          # 1960
VCHUNKS = [(0, 640), (640, 576)]                   # (col0, width)
VLO, VHI = PAD, PAD + (NCS - 1) * VSEG + H         # 32, 1920
HLO, HHI = PAD, PAD + W                            # 32, 1248


def _win_chain(nc, op, dst, t, s1, s2, F):
    """dst[n] = OP_{j=1..32} t[n-j] (left window) via doubling."""
    tt = nc.vector.tensor_tensor
    tt(s1[:, 2:F], t[:, 1:F - 1], t[:, 0:F - 2], op=op)
    tt(s2[:, 4:F], s1[:, 4:F], s1[:, 2:F - 2], op=op)
    tt(s1[:, 8:F], s2[:, 8:F], s2[:, 4:F - 4], op=op)
    tt(s2[:, 16:F], s1[:, 16:F], s1[:, 8:F - 8], op=op)
    tt(dst[:, 32:F], s2[:, 32:F], s2[:, 16:F - 16], op=op)


def _dir_pair(nc, m, p, E0, E1, t_l, t_r, u0, u1, AL, BL, AR, BR,
              TL, TR, M, s1, s2, b1, b2, lo, hi, F):
    """Both directions of one axis on [lo,hi) real region of width-F planes.
    All tiles bf16 (mask windows exact 0/1; scans keep an fp32 carry).
    Outputs: AL = awd (both dirs summed), BL = aw."""
    v = nc.vector
    _win_chain(nc, Alu.mult, M, m, b1, b2, F)
    _win_chain(nc, Alu.add, TR, p, b1, b2, F)
    # T_L[n] = M[n]*exp(W32[n]);  T_R[n] = M[n+33]*exp(-W32[n+32])
    nc.scalar.activation(s1[:, lo:hi], TR[:, lo:hi], ActF.Exp)
    nc.scalar.activation(s2[:, lo:hi], TR[:, lo + R:hi + R], ActF.Exp,
                         scale=-1.0)
    v.tensor_mul(TL[:, lo:hi], M[:, lo:hi], s1[:, lo:hi])
    v.tensor_mul(TR[:, lo:hi], M[:, lo + R + 1:hi + R + 1], s2[:, lo:hi])
    v.tensor_tensor_scan(AL[:, lo:hi], u0[:, lo - 1:hi - 1], t_l[:, lo - 1:hi - 1],
                         0.0, op0=Alu.add, op1=Alu.mult)
    v.tensor_tensor_scan(BL[:, lo:hi], E0[:, lo - 1:hi - 1], m[:, lo - 1:hi - 1],
                         0.0, op0=Alu.add, op1=Alu.mult)
    v.tensor_tensor_scan(AR[:, lo:hi][:, ::-1], u1[:, lo + 1:hi + 1][:, ::-1],
                         t_r[:, lo + 1:hi + 1][:, ::-1], 0.0,
                         op0=Alu.add, op1=Alu.mult)
    v.tensor_tensor_scan(BR[:, lo:hi][:, ::-1], E1[:, lo + 1:hi + 1][:, ::-1],
                         m[:, lo + 1:hi + 1][:, ::-1], 0.0,
                         op0=Alu.add, op1=Alu.mult)
    # corrections: X32 = X - T*shift(X, 32); M_R[n] = M_L[n+33]
    v.tensor_mul(s1[:, lo:hi], TL[:, lo:hi], AL[:, lo - R:hi - R])
    v.tensor_sub(AL[:, lo:hi], AL[:, lo:hi], s1[:, lo:hi])
    v.tensor_mul(s2[:, lo:hi], TR[:, lo:hi], AR[:, lo + R:hi + R])
    v.tensor_sub(AR[:, lo:hi], AR[:, lo:hi], s2[:, lo:hi])
    v.tensor_mul(s1[:, lo:hi], M[:, lo:hi], BL[:, lo - R:hi - R])
    v.tensor_sub(BL[:, lo:hi], BL[:, lo:hi], s1[:, lo:hi])
    v.tensor_mul(s2[:, lo:hi], M[:, lo + R + 1:hi + R + 1], BR[:, lo + R:hi + R])
    v.tensor_sub(BR[:, lo:hi], BR[:, lo:hi], s2[:, lo:hi])
    v.tensor_add(AL[:, lo:hi], AL[:, lo:hi], AR[:, lo:hi])
    v.tensor_add(BL[:, lo:hi], BL[:, lo:hi], BR[:, lo:hi])


def _transpose_out(nc, pp, ident, src, stag, c0, cw):
    """src [128, FV] transposed layout -> stag [128, (seg, W)] row-major at
    column offset c0. Full-width col-seg groups merge into one PSUM tile +
    one contiguous ACT copy."""
    ncs = (cw + 127) // 128
    for rp, (r0, hs) in enumerate(RSEGS):
        cs = 0
        while cs < ncs:
            bw = min(128, cw - cs * 128)
            fb = PAD + cs * VSEG + rp * 128
            c = rp * W + c0 + cs * 128
            ng = 0
            while (cs + ng < ncs and ng < 4
                   and min(128, cw - (cs + ng) * 128) == 128):
                ng += 1
            if ng >= 2:
                ps = pp.tile([128, 128 * ng], F16, tag="pt2", bufs=5,
                             name="psg")
                for g in range(ng):
                    nc.tensor.transpose(ps[0:hs, 128 * g:128 * (g + 1)],
                                        src[:, fb + VSEG * g:fb + VSEG * g + hs],
                                        ident[:, :])
                nc.scalar.copy(stag[0:hs, c:c + 128 * ng],
                               ps[0:hs, 0:128 * ng])
                cs += ng
            else:
                ps = pp.tile([128, 128], F16, tag="pt", bufs=3)
                nc.tensor.transpose(ps[0:hs, 0:bw], src[0:bw, fb:fb + hs],
                                    ident[0:bw, 0:bw])
                nc.scalar.copy(stag[0:hs, c:c + bw], ps[0:hs, 0:bw])
                cs += 1


def _v_phase(nc, tc, pp, ident, vcol, rmw, rmwd):
    v = nc.vector
    lo, hi = VLO, VHI
    with tc.tile_pool(name="vp", bufs=1) as vp:
        def t_(tag, bufs=1):
            return vp.tile([128, FV], F16, tag=tag, name=tag, bufs=bufs)

        mT, DT = t_("mT", 2), t_("DT", 2)
        e2, e3, pv = t_("e2", 2), t_("e3", 2), t_("pv", 2)
        tu, td, uu, ud = t_("tu"), t_("td"), t_("uu"), t_("ud")
        AL, BL, AR, BR = t_("vAL"), t_("vBL"), t_("vAR"), t_("vBR")
        TL, TR = t_("vTL"), t_("vTR")
        s1, s2 = t_("vs1"), t_("vs2")
        b1, b2, M = t_("vb1"), t_("vb2"), t_("vM")
        # one-time edge-strip zeroing (scan/correction shifted reads touch
        # these; SBUF garbage could be NaN and NaN*0 = NaN)
        for t in (tu, td, uu, ud, AL, BL, AR, BR):
            nc.gpsimd.memset(t[:, 0:lo], 0.0)
            nc.gpsimd.memset(t[:, hi:FV], 0.0)

        for c, (c0, cw) in enumerate(VCHUNKS):
            for i, t in enumerate((mT, DT, e2, e3, pv)):
                nc.sync.dma_start(t[:], vcol[i, c])
            # E = max(exp(-var), e^-CLIP)  ==  exp(-min(var, CLIP))
            nc.scalar.activation(e2[:, lo:hi], e2[:, lo:hi], ActF.Exp,
                                 scale=-1.0)
            nc.scalar.activation(e3[:, lo:hi], e3[:, lo:hi], ActF.Exp,
                                 scale=-1.0)
            v.tensor_scalar_max(e2[:, lo:hi], e2[:, lo:hi], EMIN)
            v.tensor_scalar_max(e3[:, lo:hi], e3[:, lo:hi], EMIN)
            nc.scalar.activation(s1[:, lo:hi], pv[:, lo:hi], ActF.Exp)
            nc.scalar.activation(s2[:, lo - 1:hi], pv[:, lo - 1:hi],
                                 ActF.Exp, scale=-1.0)
            v.tensor_mul(tu[:, lo:hi], mT[:, lo:hi], s1[:, lo:hi])
            v.tensor_mul(td[:, lo:hi], mT[:, lo:hi], s2[:, lo - 1:hi - 1])
            v.tensor_mul(uu[:, lo:hi], e2[:, lo:hi], DT[:, lo:hi])
            v.tensor_mul(ud[:, lo:hi], e3[:, lo:hi], DT[:, lo:hi])

            _dir_pair(nc, mT, pv, e2, e3, tu, td, uu, ud, AL, BL, AR, BR,
                      TL, TR, M, s1, s2, b1, b2, lo, hi, FV)

            _transpose_out(nc, pp, ident, AL, rmwd, c0, cw)
            _transpose_out(nc, pp, ident, BL, rmw, c0, cw)


def _h_phase(nc, tc, pp, hrow, dout, rmw, rmwd, lam_t, eps_t):
    v = nc.vector
    lo, hi = HLO, HHI
    with tc.tile_pool(name="hp", bufs=1) as hp:
        def t_(tag, bufs=1, w=FH, dt=F16):
            return hp.tile([128, w], dt, tag=tag, name=tag, bufs=bufs)

        mh, Do, Dc = t_("mh", 2), t_("Do", 2), t_("Dc", 2)
        e0, e1, ph = t_("e0", 2), t_("e1", 2), t_("ph", 2)
        tl, tr, u0, u1 = t_("tl"), t_("tr"), t_("u0"), t_("u1")
        AL, BL, AR, BR = t_("AL"), t_("BL"), t_("AR"), t_("BR")
        TL, TR = t_("TL"), t_("TR")
        s1, s2 = t_("s1"), t_("s2")
        b1, b2, M = t_("b1"), t_("b2"), t_("M")
        mlam = t_("mlam")
        tww, twdw = t_("tww", 1, W), t_("twdw", 1, W)
        sel, blo = t_("sel", 1, W), t_("blo", 1, W)
        rcl = t_("rcl", 1, W, F32)
        rcb = t_("rcb", 1, W, F32)
        for t in (tl, tr, u0, u1, AL, BL, AR, BR):
            nc.gpsimd.memset(t[:, 0:lo], 0.0)
            nc.gpsimd.memset(t[:, hi:FH], 0.0)

        for s, (r0, hs) in enumerate(RSEGS):
            for i, t in enumerate((mh, Do, Dc, e0, e1, ph)):
                nc.sync.dma_start(t[:], hrow[i, s])
            nc.scalar.activation(e0[:, lo:hi], e0[:, lo:hi], ActF.Exp,
                                 scale=-1.0)
            nc.scalar.activation(e1[:, lo:hi], e1[:, lo:hi], ActF.Exp,
                                 scale=-1.0)
            v.tensor_scalar_max(e0[:, lo:hi], e0[:, lo:hi], EMIN)
            v.tensor_scalar_max(e1[:, lo:hi], e1[:, lo:hi], EMIN)
            nc.scalar.activation(s1[:, lo:hi], ph[:, lo:hi], ActF.Exp)
            nc.scalar.activation(s2[:, lo - 1:hi], ph[:, lo - 1:hi],
                                 ActF.Exp, scale=-1.0)
            v.tensor_mul(tl[:, lo:hi], mh[:, lo:hi], s1[:, lo:hi])
            v.tensor_mul(tr[:, lo:hi], mh[:, lo:hi], s2[:, lo - 1:hi - 1])
            v.tensor_mul(u0[:, lo:hi], e0[:, lo:hi], Dc[:, lo:hi])
            v.tensor_mul(u1[:, lo:hi], e1[:, lo:hi], Dc[:, lo:hi])
            nc.scalar.activation(mlam[:, lo:hi], mh[:, lo:hi], ActF.Copy,
                                 scale=lam_t[:, 0:1])

            _dir_pair(nc, mh, ph, e0, e1, tl, tr, u0, u1, AL, BL, AR, BR,
                      TL, TR, M, s1, s2, b1, b2, lo, hi, FH)

            # fused final blend, directly on SBUF tiles + V results
            for c, (c0, cw) in enumerate(VCHUNKS):
                hc = slice(lo + c0, lo + c0 + cw)
                rs_ = slice(s * W + c0, s * W + c0 + cw)
                bs = slice(c0, c0 + cw)
                v.tensor_add(tww[:, bs], BL[:, hc], rmw[:, rs_])
                v.tensor_add(twdw[:, bs], AL[:, hc], rmwd[:, rs_])
                v.tensor_scalar(sel[:, bs], tww[:, bs], 0.0, None,
                                op0=Alu.is_gt)
                v.tensor_mul(sel[:, bs], sel[:, bs], mlam[:, hc])
                # 1/tw via exp(-ln(tw + 1e-6)); ln kept in f32 for accuracy
                nc.scalar.activation(rcl[:, bs], tww[:, bs], ActF.Ln,
                                     bias=eps_t[:, 0:1])
                nc.scalar.activation(rcb[:, bs], rcl[:, bs], ActF.Exp,
                                     scale=-1.0)
                v.tensor_mul(blo[:, bs], twdw[:, bs], rcb[:, bs])
                v.tensor_sub(blo[:, bs], blo[:, bs], Do[:, hc])
                v.tensor_mul(blo[:, bs], blo[:, bs], sel[:, bs])
                v.tensor_add(blo[:, bs], blo[:, bs], Do[:, hc])
                nc.sync.dma_start(dout[s, 0:hs, bs], blo[0:hs, bs])


def build_program():
    nc = bacc.Bacc("TRN2", target_bir_lowering=False, debug=False)

    hrow = nc.dram_tensor("hrow", [6, 3, 128, FH], F16,
                          kind="ExternalInput").ap()
    vcol = nc.dram_tensor("vcol", [5, 2, 128, FV], F16,
                          kind="ExternalInput").ap()
    lam = nc.dram_tensor("lam", [1], F32, kind="ExternalInput").ap()
    dout = nc.dram_tensor("dout", [3, 128, W], F16,
                          kind="ExternalOutput").ap()

    with tile.TileContext(nc, pool_alloc_mode="queue") as tc:
        with tc.tile_pool(name="const", bufs=1) as cp, \
             tc.tile_pool(name="psum", bufs=8, space="PSUM") as pp, \
             tc.tile_pool(name="persist", bufs=1) as qp:
            ident = cp.tile([128, 128], F16, tag="ident")
            masks.make_identity(nc, ident[:])
            lam_t = cp.tile([128, 1], F32, tag="lam")
            nc.sync.dma_start(lam_t[:, 0:1], lam.partition_broadcast(128))
            eps_t = cp.tile([128, 1], F32, tag="eps")
            nc.gpsimd.memset(eps_t[:], 1e-6)
            rmw = qp.tile([128, 3 * W], F16, tag="rmw")
            rmwd = qp.tile([128, 3 * W], F16, tag="rmwd")

            _v_phase(nc, tc, pp, ident, vcol, rmw, rmwd)
            _h_phase(nc, tc, pp, hrow, dout, rmw, rmwd, lam_t, eps_t)
    nc.finalize()
    return nc


def _pack_inputs(pred_log, maskf, variance, dorig, dcur):
    """Host-side layout prep: row-major segmented planes for the H phase and
    transposed column-chunk planes for the V phase, pads zeroed, bf16."""
    nb = maskf.shape[0]
    planes = np.stack([maskf, dorig, dcur,
                       variance[:, 0], variance[:, 1], pred_log[:, 0]], 1)
    pb = planes.astype(NF16)
    hrow = np.zeros((nb, 6, 3, 128, FH), NF16)
    for s, (r0, hs) in enumerate(RSEGS):
        hrow[:, :, s, 0:hs, PAD:PAD + W] = pb[:, :, r0:r0 + hs, :]
    vplanes = np.stack([maskf, dcur, variance[:, 2], variance[:, 3],
                        pred_log[:, 1]], 1)
    vT = np.ascontiguousarray(vplanes.transpose(0, 1, 3, 2)).astype(NF16)
    vcol = np.zeros((nb, 5, 2, 128, FV), NF16)
    for c, (c0, cw) in enumerate(VCHUNKS):
        for s in range(NCS):
            bw = min(128, cw - s * 128)
            w0 = c0 + s * 128
            vcol[:, :, c, 0:bw, PAD + s * VSEG:PAD + s * VSEG + H] = \
                vT[:, :, w0:w0 + bw, :]
    return hrow, vcol


def _unpack(dout):
    """dout [3, 128, W] bf16 -> [H, W] f32."""
    return np.concatenate(
        [np.asarray(dout[s][0:hs], np.float32)
         for s, (r0, hs) in enumerate(RSEGS)], axis=0)


_NC = None


def _get_nc():
    global _NC
    if _NC is None:
        _NC = build_program()
    return _NC


def kernel(pred_log, mask, variance, depthin, lam, times):
    pred_log = np.asarray(pred_log, np.float32)
    mask = np.asarray(mask, np.int32)
    variance = np.asarray(variance, np.float32)
    depthin = np.asarray(depthin, np.float32)
    lam = np.asarray(lam, np.float32).reshape(1)
    t = int(np.asarray(times))

    if t <= 0:
        return depthin.copy()
    nc = _get_nc()
    maskf = mask[:, 0].astype(np.float32)
    dorig = depthin[:, 0]
    dcur = dorig
    for _ in range(t):
        hrow, vcol = _pack_inputs(pred_log, maskf, variance, dorig, dcur)
        in_maps = [{"hrow": hrow[b], "vcol": vcol[b], "lam": lam}
                   for b in range(B)]
        res = run_bass_kernel_spmd(nc, in_maps, list(range(B)))
        dcur = np.stack([_unpack(res.results[i]["dout"]) for i in range(B)])
    return dcur[:, None].astype(np.float32)


# revision 6
# speedup vs baseline: 1.9775x; 1.0877x over previous
"""CRF integration (nn_CRFIntegrationModule) Trainium2 kernel — v2.

One image per NeuronCore (B=8 -> 8 cores). Each direction's 32-step scan is
a hardware tensor_tensor_scan (fp32 carry, bf16 i/o) plus a windowed
correction:

    A_inf[n] = (A_inf[n-1] + u[n-1]) * t[n-1]        (one DVE scan op)
    T_32[n]  = prod_{j=1..32} t[n-j]                 (5 doubling multiplies)
    A_32     = A_inf - T_32 * shift(A_inf, 32)       (exact windowed sum)

v2 layout strategy: the host pre-packs BOTH layouts (row-major segments for
the horizontal scans, transposed column-chunks for the vertical scans) as
bf16 DRAM tensors with pads baked in, so the device does no staging
transposes on the input side.  All elementwise work runs in bf16 (2x DVE
tensor_tensor, 4x tensor_scalar); only the V-phase results are transposed
back on-chip (PE + ACT), and the final blend reads the H-phase tiles
directly from SBUF.  tolerance budget 2e-2 >> bf16 rounding.
"""
import os
import sys

for _p in ("/opt/trn_rl_repo", "/root/.axon_site/_ro/trn_rl_repo"):
    if os.path.isdir(_p) and _p not in sys.path:
        sys.path.insert(0, _p)
        break

import numpy as np
import ml_dtypes
import concourse.bacc as bacc
import concourse.mybir as mybir
import concourse.tile as tile
from concourse import masks
from concourse.bass_utils import run_bass_kernel_spmd

Alu = mybir.AluOpType
ActF = mybir.ActivationFunctionType
F32 = mybir.dt.float32
F16 = mybir.dt.float16
NF16 = np.float16

B, H, W = 8, 352, 1216
R = 32          # MAXRANGE
CLIP = 5.0      # CLIPVARIANCE
PAD = 32
EMIN = float(np.exp(-CLIP))

# H-phase geometry: row segments (partitions = rows), row-major free axis
RSEGS = [(0, 128), (128, 128), (256, 96)]
FH = W + 2 * PAD + 8                               # 1288 (+8: M[n+33] slack)

# V-phase geometry: transposed layout, 2 chunks x 5 col-segments of <=128 cols
VSEG = H + PAD                                     # 384 per col-seg span
NCS = 5
FV = PAD + NCS * VSEG + 8                          # 1960
VCHUNKS = [(0, 640), (640, 576)]                   # (col0, width)
VLO, VHI = PAD, PAD + (NCS - 1) * VSEG + H         # 32, 1920
HLO, HHI = PAD, PAD + W                            # 32, 1248


def _win_chain_g(nc, dst, t, g1, g2, F):
    """dst[n] = prod_{j=1..32} t[n-j] via doubling, on the (idle) Pool
    engine.  Exact for 0/1 masks in fp16."""
    gt = nc.gpsimd.tensor_tensor
    gt(g1[:, 2:F], t[:, 1:F - 1], t[:, 0:F - 2], op=Alu.mult)
    gt(g2[:, 4:F], g1[:, 4:F], g1[:, 2:F - 2], op=Alu.mult)
    gt(g1[:, 8:F], g2[:, 8:F], g2[:, 4:F - 4], op=Alu.mult)
    gt(g2[:, 16:F], g1[:, 16:F], g1[:, 8:F - 8], op=Alu.mult)
    gt(dst[:, 32:F], g2[:, 32:F], g2[:, 16:F - 16], op=Alu.mult)


def _dir_pair(nc, m, p, E0, E1, t_l, t_r, u0, u1, AL, BL, AR, BR,
              TL, TR, M, s1, s2, g1, g2, CP, lo, hi, F):
    """Both directions of one axis on [lo,hi) real region of width-F planes.
    fp16 tiles (mask windows exact 0/1; scans keep an fp32 carry).
    M = windowed mask product on Pool; W32 = windowed plog sum via an
    exclusive f32 prefix scan + shifted difference (exact integers / tiny
    sums, no cancellation issue).  Outputs: AL = awd, BL = aw."""
    v = nc.vector
    _win_chain_g(nc, M, m, g1, g2, F)
    hiW = hi + R
    v.tensor_tensor_scan(CP[:, 1:hiW], p[:, 0:hiW - 1], p[:, 0:hiW - 1],
                         0.0, op0=Alu.add, op1=Alu.bypass)
    v.scalar_tensor_tensor(TR[:, lo:hiW], CP[:, lo:hiW], 0.0,
                           CP[:, lo - R:hi], op0=Alu.bypass,
                           op1=Alu.subtract)
    # T_L[n] = M[n]*exp(W32[n]);  T_R[n] = M[n+33]*exp(-W32[n+32])
    nc.scalar.activation(s1[:, lo:hi], TR[:, lo:hi], ActF.Exp)
    nc.scalar.activation(s2[:, lo:hi], TR[:, lo + R:hi + R], ActF.Exp,
                         scale=-1.0)
    v.tensor_mul(TL[:, lo:hi], M[:, lo:hi], s1[:, lo:hi])
    v.tensor_mul(TR[:, lo:hi], M[:, lo + R + 1:hi + R + 1], s2[:, lo:hi])
    v.tensor_tensor_scan(AL[:, lo:hi], u0[:, lo - 1:hi - 1], t_l[:, lo - 1:hi - 1],
                         0.0, op0=Alu.add, op1=Alu.mult)
    v.tensor_tensor_scan(BL[:, lo:hi], E0[:, lo - 1:hi - 1], m[:, lo - 1:hi - 1],
                         0.0, op0=Alu.add, op1=Alu.mult)
    v.tensor_tensor_scan(AR[:, lo:hi][:, ::-1], u1[:, lo + 1:hi + 1][:, ::-1],
                         t_r[:, lo + 1:hi + 1][:, ::-1], 0.0,
                         op0=Alu.add, op1=Alu.mult)
    v.tensor_tensor_scan(BR[:, lo:hi][:, ::-1], E1[:, lo + 1:hi + 1][:, ::-1],
                         m[:, lo + 1:hi + 1][:, ::-1], 0.0,
                         op0=Alu.add, op1=Alu.mult)
    # corrections: X32 = X - T*shift(X, 32); M_R[n] = M_L[n+33]
    v.tensor_mul(s1[:, lo:hi], TL[:, lo:hi], AL[:, lo - R:hi - R])
    v.tensor_sub(AL[:, lo:hi], AL[:, lo:hi], s1[:, lo:hi])
    v.tensor_mul(s2[:, lo:hi], TR[:, lo:hi], AR[:, lo + R:hi + R])
    v.tensor_sub(AR[:, lo:hi], AR[:, lo:hi], s2[:, lo:hi])
    v.tensor_mul(s1[:, lo:hi], M[:, lo:hi], BL[:, lo - R:hi - R])
    v.tensor_sub(BL[:, lo:hi], BL[:, lo:hi], s1[:, lo:hi])
    v.tensor_mul(s2[:, lo:hi], M[:, lo + R + 1:hi + R + 1], BR[:, lo + R:hi + R])
    v.tensor_sub(BR[:, lo:hi], BR[:, lo:hi], s2[:, lo:hi])
    v.tensor_add(AL[:, lo:hi], AL[:, lo:hi], AR[:, lo:hi])
    v.tensor_add(BL[:, lo:hi], BL[:, lo:hi], BR[:, lo:hi])


def _transpose_out(nc, pp, ident, src, stag, c0, cw):
    """src [128, FV] transposed layout -> stag [128, (seg, W)] row-major at
    column offset c0. Full-width col-seg groups merge into one PSUM tile +
    one contiguous ACT copy."""
    ncs = (cw + 127) // 128
    for rp, (r0, hs) in enumerate(RSEGS):
        cs = 0
        while cs < ncs:
            bw = min(128, cw - cs * 128)
            fb = PAD + cs * VSEG + rp * 128
            c = rp * W + c0 + cs * 128
            ng = 0
            while (cs + ng < ncs and ng < 4
                   and min(128, cw - (cs + ng) * 128) == 128):
                ng += 1
            if ng >= 2:
                ps = pp.tile([128, 128 * ng], F16, tag="pt2", bufs=5,
                             name="psg")
                for g in range(ng):
                    nc.tensor.transpose(ps[0:hs, 128 * g:128 * (g + 1)],
                                        src[:, fb + VSEG * g:fb + VSEG * g + hs],
                                        ident[:, :])
                nc.scalar.copy(stag[0:hs, c:c + 128 * ng],
                               ps[0:hs, 0:128 * ng])
                cs += ng
            else:
                ps = pp.tile([128, 128], F16, tag="pt", bufs=3)
                nc.tensor.transpose(ps[0:hs, 0:bw], src[0:bw, fb:fb + hs],
                                    ident[0:bw, 0:bw])
                nc.scalar.copy(stag[0:hs, c:c + bw], ps[0:hs, 0:bw])
                cs += 1


def _v_phase(nc, tc, pp, ident, vcol, rmw, rmwd):
    v = nc.vector
    lo, hi = VLO, VHI
    with tc.tile_pool(name="vp", bufs=1) as vp:
        def t_(tag, bufs=1):
            return vp.tile([128, FV], F16, tag=tag, name=tag, bufs=bufs)

        mT, DT = t_("mT", 2), t_("DT", 2)
        e2, e3, pv = t_("e2", 2), t_("e3", 2), t_("pv", 2)
        tu, td, uu, ud = t_("tu"), t_("td"), t_("uu"), t_("ud")
        AL, BL, AR, BR = t_("vAL"), t_("vBL"), t_("vAR"), t_("vBR")
        TL, TR = t_("vTL"), t_("vTR")
        s1, s2 = t_("vs1"), t_("vs2")
        g1, g2, M = t_("vg1"), t_("vg2"), t_("vM")
        CP = vp.tile([128, FV], F32, tag="vCP", name="vCP")
        nc.gpsimd.memset(CP[:, 0:1], 0.0)
        # one-time edge-strip zeroing (scan/correction shifted reads touch
        # these; SBUF garbage could be NaN and NaN*0 = NaN)
        for t in (tu, td, uu, ud, AL, BL, AR, BR):
            nc.gpsimd.memset(t[:, 0:lo], 0.0)
            nc.gpsimd.memset(t[:, hi:FV], 0.0)

        for c, (c0, cw) in enumerate(VCHUNKS):
            for i, t in enumerate((mT, DT, e2, e3, pv)):
                nc.sync.dma_start(t[:], vcol[i, c])
            # E = max(exp(-var), e^-CLIP)  ==  exp(-min(var, CLIP))
            nc.scalar.activation(e2[:, lo:hi], e2[:, lo:hi], ActF.Exp,
                                 scale=-1.0)
            nc.scalar.activation(e3[:, lo:hi], e3[:, lo:hi], ActF.Exp,
                                 scale=-1.0)
            v.tensor_scalar_max(e2[:, lo:hi], e2[:, lo:hi], EMIN)
            v.tensor_scalar_max(e3[:, lo:hi], e3[:, lo:hi], EMIN)
            nc.scalar.activation(s1[:, lo:hi], pv[:, lo:hi], ActF.Exp)
            nc.scalar.activation(s2[:, lo - 1:hi], pv[:, lo - 1:hi],
                                 ActF.Exp, scale=-1.0)
            v.tensor_mul(tu[:, lo:hi], mT[:, lo:hi], s1[:, lo:hi])
            v.tensor_mul(td[:, lo:hi], mT[:, lo:hi], s2[:, lo - 1:hi - 1])
            v.tensor_mul(uu[:, lo:hi], e2[:, lo:hi], DT[:, lo:hi])
            nc.gpsimd.tensor_mul(ud[:, lo:hi], e3[:, lo:hi], DT[:, lo:hi])

            _dir_pair(nc, mT, pv, e2, e3, tu, td, uu, ud, AL, BL, AR, BR,
                      TL, TR, M, s1, s2, g1, g2, CP, lo, hi, FV)

            _transpose_out(nc, pp, ident, AL, rmwd, c0, cw)
            _transpose_out(nc, pp, ident, BL, rmw, c0, cw)


def _h_phase(nc, tc, pp, hrow, dout, rmw, rmwd, lam_t, eps_t):
    v = nc.vector
    lo, hi = HLO, HHI
    with tc.tile_pool(name="hp", bufs=1) as hp:
        def t_(tag, bufs=1, w=FH, dt=F16):
            return hp.tile([128, w], dt, tag=tag, name=tag, bufs=bufs)

        mh, Do, Dc = t_("mh", 2), t_("Do", 2), t_("Dc", 2)
        e0, e1, ph = t_("e0", 2), t_("e1", 2), t_("ph", 2)
        tl, tr, u0, u1 = t_("tl"), t_("tr"), t_("u0"), t_("u1")
        AL, BL, AR, BR = t_("AL"), t_("BL"), t_("AR"), t_("BR")
        TL, TR = t_("TL"), t_("TR")
        s1, s2 = t_("s1"), t_("s2")
        g1, g2, M = t_("g1"), t_("g2"), t_("M")
        CP = hp.tile([128, FH], F32, tag="hCP", name="hCP")
        nc.gpsimd.memset(CP[:, 0:1], 0.0)
        mlam = t_("mlam")
        tww, twdw = t_("tww", 1, W), t_("twdw", 1, W)
        sel, blo = t_("sel", 1, W), t_("blo", 1, W)
        rcl = t_("rcl", 1, W, F32)
        rcb = t_("rcb", 1, W, F32)
        for t in (tl, tr, u0, u1, AL, BL, AR, BR):
            nc.gpsimd.memset(t[:, 0:lo], 0.0)
            nc.gpsimd.memset(t[:, hi:FH], 0.0)

        for s, (r0, hs) in enumerate(RSEGS):
            for i, t in enumerate((mh, Do, Dc, e0, e1, ph)):
                nc.sync.dma_start(t[:], hrow[i, s])
            nc.scalar.activation(e0[:, lo:hi], e0[:, lo:hi], ActF.Exp,
                                 scale=-1.0)
            nc.scalar.activation(e1[:, lo:hi], e1[:, lo:hi], ActF.Exp,
                                 scale=-1.0)
            v.tensor_scalar_max(e0[:, lo:hi], e0[:, lo:hi], EMIN)
            v.tensor_scalar_max(e1[:, lo:hi], e1[:, lo:hi], EMIN)
            nc.scalar.activation(s1[:, lo:hi], ph[:, lo:hi], ActF.Exp)
            nc.scalar.activation(s2[:, lo - 1:hi], ph[:, lo - 1:hi],
                                 ActF.Exp, scale=-1.0)
            v.tensor_mul(tl[:, lo:hi], mh[:, lo:hi], s1[:, lo:hi])
            v.tensor_mul(tr[:, lo:hi], mh[:, lo:hi], s2[:, lo - 1:hi - 1])
            v.tensor_mul(u0[:, lo:hi], e0[:, lo:hi], Dc[:, lo:hi])
            nc.gpsimd.tensor_mul(u1[:, lo:hi], e1[:, lo:hi], Dc[:, lo:hi])
            nc.scalar.activation(mlam[:, lo:hi], mh[:, lo:hi], ActF.Copy,
                                 scale=lam_t[:, 0:1])

            _dir_pair(nc, mh, ph, e0, e1, tl, tr, u0, u1, AL, BL, AR, BR,
                      TL, TR, M, s1, s2, g1, g2, CP, lo, hi, FH)

            # fused final blend, directly on SBUF tiles + V results
            for c, (c0, cw) in enumerate(VCHUNKS):
                hc = slice(lo + c0, lo + c0 + cw)
                rs_ = slice(s * W + c0, s * W + c0 + cw)
                bs = slice(c0, c0 + cw)
                v.tensor_add(tww[:, bs], BL[:, hc], rmw[:, rs_])
                v.tensor_add(twdw[:, bs], AL[:, hc], rmwd[:, rs_])
                v.tensor_scalar(sel[:, bs], tww[:, bs], 0.0, None,
                                op0=Alu.is_gt)
                v.tensor_mul(sel[:, bs], sel[:, bs], mlam[:, hc])
                # 1/tw via exp(-ln(tw + 1e-6)); ln kept in f32 for accuracy
                nc.scalar.activation(rcl[:, bs], tww[:, bs], ActF.Ln,
                                     bias=eps_t[:, 0:1])
                nc.scalar.activation(rcb[:, bs], rcl[:, bs], ActF.Exp,
                                     scale=-1.0)
                v.tensor_mul(blo[:, bs], twdw[:, bs], rcb[:, bs])
                v.tensor_sub(blo[:, bs], blo[:, bs], Do[:, hc])
                v.tensor_mul(blo[:, bs], blo[:, bs], sel[:, bs])
                v.tensor_add(blo[:, bs], blo[:, bs], Do[:, hc])
                nc.sync.dma_start(dout[s, 0:hs, bs], blo[0:hs, bs])


def build_program():
    nc = bacc.Bacc("TRN2", target_bir_lowering=False, debug=False)

    hrow = nc.dram_tensor("hrow", [6, 3, 128, FH], F16,
                          kind="ExternalInput").ap()
    vcol = nc.dram_tensor("vcol", [5, 2, 128, FV], F16,
                          kind="ExternalInput").ap()
    lam = nc.dram_tensor("lam", [1], F32, kind="ExternalInput").ap()
    dout = nc.dram_tensor("dout", [3, 128, W], F16,
                          kind="ExternalOutput").ap()

    # Pin Exp/Ln to the one ACT table set containing both, so the
    # activation-table loader emits a single LoadActFuncSet instead of
    # ping-ponging between exp-only and ln-only sets (1.3us per reload).
    from concourse.hw_specs import get_activation_tables
    _tabs = get_activation_tables(nc.m.arch)
    for _name, _s in _tabs.items():
        if _name != "natural_log_exp_and_others":
            _s.discard(ActF.Exp)
            _s.discard(ActF.Ln)

    with tile.TileContext(nc, pool_alloc_mode="queue") as tc:
        with tc.tile_pool(name="const", bufs=1) as cp, \
             tc.tile_pool(name="psum", bufs=8, space="PSUM") as pp, \
             tc.tile_pool(name="persist", bufs=1) as qp:
            ident = cp.tile([128, 128], F16, tag="ident")
            masks.make_identity(nc, ident[:])
            lam_t = cp.tile([128, 1], F32, tag="lam")
            nc.sync.dma_start(lam_t[:, 0:1], lam.partition_broadcast(128))
            eps_t = cp.tile([128, 1], F32, tag="eps")
            nc.gpsimd.memset(eps_t[:], 1e-6)
            rmw = qp.tile([128, 3 * W], F16, tag="rmw")
            rmwd = qp.tile([128, 3 * W], F16, tag="rmwd")

            _v_phase(nc, tc, pp, ident, vcol, rmw, rmwd)
            _h_phase(nc, tc, pp, hrow, dout, rmw, rmwd, lam_t, eps_t)
    nc.finalize()
    return nc


def _pack_inputs(pred_log, maskf, variance, dorig, dcur):
    """Host-side layout prep: row-major segmented planes for the H phase and
    transposed column-chunk planes for the V phase, pads zeroed, bf16."""
    nb = maskf.shape[0]
    planes = np.stack([maskf, dorig, dcur,
                       variance[:, 0], variance[:, 1], pred_log[:, 0]], 1)
    pb = planes.astype(NF16)
    hrow = np.zeros((nb, 6, 3, 128, FH), NF16)
    for s, (r0, hs) in enumerate(RSEGS):
        hrow[:, :, s, 0:hs, PAD:PAD + W] = pb[:, :, r0:r0 + hs, :]
    vplanes = np.stack([maskf, dcur, variance[:, 2], variance[:, 3],
                        pred_log[:, 1]], 1)
    vT = np.ascontiguousarray(vplanes.transpose(0, 1, 3, 2)).astype(NF16)
    vcol = np.zeros((nb, 5, 2, 128, FV), NF16)
    for c, (c0, cw) in enumerate(VCHUNKS):
        for s in range(NCS):
            bw = min(128, cw - s * 128)
            w0 = c0 + s * 128
            vcol[:, :, c, 0:bw, PAD + s * VSEG:PAD + s * VSEG + H] = \
                vT[:, :, w0:w0 + bw, :]
    return hrow, vcol


def _unpack(dout):
    """dout [3, 128, W] bf16 -> [H, W] f32."""
    return np.concatenate(
        [np.asarray(dout[s][0:hs], np.float32)
         for s, (r0, hs) in enumerate(RSEGS)], axis=0)


_NC = None


def _get_nc():
    global _NC
    if _NC is None:
        _NC = build_program()
    return _NC


def kernel(pred_log, mask, variance, depthin, lam, times):
    pred_log = np.asarray(pred_log, np.float32)
    mask = np.asarray(mask, np.int32)
    variance = np.asarray(variance, np.float32)
    depthin = np.asarray(depthin, np.float32)
    lam = np.asarray(lam, np.float32).reshape(1)
    t = int(np.asarray(times))

    if t <= 0:
        return depthin.copy()
    nc = _get_nc()
    maskf = mask[:, 0].astype(np.float32)
    dorig = depthin[:, 0]
    dcur = dorig
    for _ in range(t):
        hrow, vcol = _pack_inputs(pred_log, maskf, variance, dorig, dcur)
        in_maps = [{"hrow": hrow[b], "vcol": vcol[b], "lam": lam}
                   for b in range(B)]
        res = run_bass_kernel_spmd(nc, in_maps, list(range(B)))
        dcur = np.stack([_unpack(res.results[i]["dout"]) for i in range(B)])
    return dcur[:, None].astype(np.float32)
